# revision 1
# baseline (speedup 1.0000x reference)
"""Trainium2 Bass kernel for nn_Attention_31396210933853.

Computation (B=32, S=4096, D=512):
    eij[b,s] = sum_d x[b,s,d]*kernel[d] + bias[s]
    a        = exp(tanh(eij)) * mask
    out[b,d] = sum_s a[b,s]*x[b,s,d] / (sum_s a[b,s] + EPS)

Single pass over x (normalization deferred): U = sum a*x and den =
sum a accumulate together, out = U/(den+EPS).  x is read from HBM
exactly once -> memory-bound.  Measured: solo DMA sustains ~416 GB/s
per core; 8 cores together contend for chip HBM, so a core sees
335-400 GB/s (32 MiB/core => 84-98us window).

Sharding: data-parallel over batch, 4 samples per core on 8 cores.

Layout: per sample, S=4096 splits into 2 mega-tiles of 2048 positions;
mega-tile (128, 16*512) holds s = mg*2048 + p*16 + j' at partition p,
free offset j'*512+d.  Each mega is consumed as two 8-column groups;
a group normally loads as ONE dma_start (16 KiB/partition descriptors
-> best DMA efficiency), the pipeline head/tail groups split into 2/4
smaller loads to shorten first/last-chain latency.  Per group:
  DVE : 6 scalar_tensor_tensor (x*k, fused free-dim reduce via
        accum_out, kernel operand read from PSUM to keep SBUF ports
        free for the x DMA) -> eraw columns; + bias (one (128,8) op)
  GpS : 2 tensor_mul columns (no STT on Pool) handed to
  ACT : Copy+accum reduce; tanh, exp (batched (128,8))
  GpS : * mask -> a (128,8) in fp32r
  PE  : 8 matmuls a_j^T @ x_seg_j -> U psum (1,512)/sample (fp32r,
        1 cyc/row) + ones^T @ a -> den column slice (start=stop).
Finalize per sample (emitted 2 groups into the next sample so the
in-order DVE/ACT queues never stall on the PE counter): den reduce on
ACT, EPS+reciprocal on DVE, U*rec on ACT, out DMA on the scalar ring
(the sync ring is reserved for x loads: anything else enqueued there
head-of-line-blocks later x dma_starts).

Pipeline-shape notes (measured on HW): per-group batching of the
pointwise chain removes the tiny-op overhead that throttled the
per-tile version; XBUFS=5 bounds in-flight DMA, which both fits SBUF
and keeps cross-core HBM arbitration fair; deeper prefetch or a
second issuing ring makes the 8-core ensemble slower.
"""
import numpy as np

import concourse.bass as bass
import concourse.bacc as bacc
import concourse.tile as tile
from concourse import mybir
from concourse.bass_utils import run_bass_kernel_spmd

B, S, D = 32, 4096, 512
N_CORES = 8
BC = B // N_CORES        # samples per core
P = 128                  # SBUF partitions
GRP = 8                  # s-columns per group (one activation batch)
NG = S // (P * GRP)      # groups per sample (4)
MEGA = 2                 # groups per host mega-tile: 16 KiB/partition DMAs
EPS = 1e-7

# j-columns computed on GpSimd+ACT (rest on DVE). Half 0 owns j 0-3,
# half 1 owns j 4-7. GpSimd has no scalar_tensor_tensor on TRN2, so its
# columns cost a GpSimd multiply + ACT accumulate (~2x a DVE STT) ->
# give it fewer columns; alternate 2/3 per group to average 2.5 so the
# DVE stays just under the DMA rate.
GPS_J_EVEN = (3, 7)
GPS_J_ODD = (3, 7)
XBUFS = 5                # group tile pipeline depth (16 KiB/partition each)

PASS_B_FP32R = True

# Set by a driver (e.g. test harness) to profile; off by default.
TRACE = False
LAST_RESULTS = None

_PROGRAM_CACHE = {}


def _build_program(fp32r: bool):
    f32 = mybir.dt.float32
    f32r = mybir.dt.float32r
    FT = mybir.ActivationFunctionType
    OP = mybir.AluOpType

    nc = bacc.Bacc(
        "TRN2", target_bir_lowering=False, debug=False, num_devices=N_CORES
    )
    xdt = f32r if fp32r else f32
    x_d = nc.dram_tensor(
        "x", [BC, NG // MEGA, P, MEGA * GRP * D], xdt, kind="ExternalInput"
    )
    kb_d = nc.dram_tensor("kb", [1, D], f32, kind="ExternalInput")
    bias_d = nc.dram_tensor("bias_t", [P, NG * GRP], f32, kind="ExternalInput")
    mask_d = nc.dram_tensor("mask_t", [BC, P, NG * GRP], f32, kind="ExternalInput")
    ones_d = nc.dram_tensor("ones", [P, 1], xdt, kind="ExternalInput")
    out_d = nc.dram_tensor("out", [1, BC * D], f32, kind="ExternalOutput")

    with tile.TileContext(nc) as tc:
        with (
            tc.tile_pool(name="xp", bufs=XBUFS) as xp,
            tc.tile_pool(name="cons", bufs=1) as cons,
            tc.tile_pool(name="tmpd", bufs=3) as tmpd,
            tc.tile_pool(name="tmpg", bufs=2) as tmpg,
            tc.tile_pool(name="tmpa", bufs=2) as tmpa,
            tc.tile_pool(name="small", bufs=8) as small,
            tc.tile_pool(name="fin", bufs=4) as fin,
            tc.tile_pool(name="psum", bufs=1, space="PSUM") as psp,
        ):
            # kb rides the sync HWDGE ring so the first STT isn't gated
            # on the slower SWDGE path; _emit_kb() is called after the
            # first two x dma_starts so the x stream starts ~1.4us
            # earlier (kb still lands well before the first STT needs
            # it).  The DVE's copy lives in PSUM: its 6 reads/group
            # would otherwise compete with the x DMA for SBUF bandwidth
            # (DMA measures 416 GB/s standalone vs ~334 GB/s under
            # compute load).  GpSimd cannot access PSUM -> SBUF copy.
            kb_sb = cons.tile([P, D], f32)
            kb = psp.tile([P, D], f32, tag="kb")

            def _emit_kb():
                nc.sync.dma_start(out=kb_sb, in_=kb_d.ap().to_broadcast([P, D]))
                nc.scalar.copy(kb, kb_sb)
            bias_t = cons.tile([P, NG * GRP], f32)
            nc.gpsimd.dma_start(out=bias_t, in_=bias_d[:])
            mask_all = cons.tile([P, BC * NG * GRP], f32)
            for b in range(BC):
                nc.gpsimd.dma_start(
                    out=mask_all[:, b * NG * GRP : (b + 1) * NG * GRP],
                    in_=mask_d[b],
                )
            ones = cons.tile([P, 1], xdt)
            nc.gpsimd.dma_start(out=ones, in_=ones_d[:])
            out_row = cons.tile([1, BC * D], f32)

            u_ps = [
                psp.tile([1, D], f32, name=f"u_ps{b}", tag=f"u{b}")
                for b in range(BC)
            ]
            den_ps = psp.tile([1, BC * NG * GRP], f32, tag="den")

            def _finalize(b):
                # Runs well after sample b's last matmul (emission is
                # deferred into the next sample) so the in-order DVE queue
                # never stalls on the PE counter.  The heavy pieces (den
                # reduce, U*rec) ride the ACT engine, keeping the DVE free
                # for STTs; the out DMA rides the scalar ring: putting it
                # on the sync ring would block later x-tile dma_starts
                # behind the whole sample pipeline.
                dend = fin.tile([1, NG * GRP], f32, tag="dend", name=f"dend{b}")
                denr = fin.tile([1, 1], f32, tag="denr", name=f"denr{b}")
                nc.scalar.activation(
                    dend,
                    den_ps[:, b * NG * GRP : (b + 1) * NG * GRP],
                    FT.Copy,
                    accum_out=denr,
                )
                deno = fin.tile([1, 1], f32, tag="deno", name=f"deno{b}")
                nc.vector.tensor_scalar_add(deno, denr, EPS)
                rec = fin.tile([1, 1], f32, tag="rec", name=f"rec{b}")
                nc.vector.reciprocal(rec, deno)
                nc.scalar.mul(out_row[:, b * D : (b + 1) * D], u_ps[b], rec)
                nc.scalar.dma_start(
                    out=out_d[:, b * D : (b + 1) * D],
                    in_=out_row[:, b * D : (b + 1) * D],
                )

            # Zero-bias AP for activations: a float bias would pull in the
            # per-engine const-scalar table load in the preamble.
            zero_b = cons.tile([P, 1], f32)
            nc.scalar.memzero(zero_b)

            def emit_group(b, g, gps_j, n_chains, n_dma=1):
                # One 16 KiB/partition load per group by default (larger
                # descriptors run ~2% faster); n_dma=2 splits it into two
                # 8 KiB loads so the first chain can start half a group
                # earlier (used at the pipeline head and tail).
                mg, off = divmod(g, MEGA)
                seg = GRP * D // n_dma
                xh = [
                    xp.tile([P, seg], xdt, name=f"xh{n_dma}_{h}", tag="xh")
                    for h in range(n_dma)
                ]
                for h in range(n_dma):
                    o = off * GRP * D + h * seg
                    nc.sync.dma_start(
                        out=xh[h], in_=x_d[b, mg][:, o : o + seg]
                    )
                if b == 0 and g == 0:
                    _emit_kb()

                cpt = GRP // n_dma  # columns per dma tile
                eraw = small.tile([P, GRP], f32)
                for j in range(GRP):
                    h, jj = divmod(j, cpt)
                    src = xh[h] if fp32r is False else xh[h].bitcast(f32)
                    if j in gps_j:
                        # Pool engine lacks scalar_tensor_tensor and
                        # free-axis tensor_reduce: multiply on GpSimd,
                        # reduce via ACT's accumulator (Copy+accum).
                        tmp = tmpg.tile([P, D], f32, name="tmpg", tag="tg")
                        nc.gpsimd.tensor_mul(
                            tmp, src[:, jj * D : (jj + 1) * D], kb_sb
                        )
                        tmp2 = tmpa.tile([P, D], f32, name="tmpa", tag="ta")
                        nc.scalar.activation(
                            tmp2,
                            tmp,
                            FT.Copy,
                            accum_out=eraw[:, j : j + 1],
                        )
                    else:
                        tmp = tmpd.tile([P, D], f32, name="tmpd", tag="td")
                        nc.vector.scalar_tensor_tensor(
                            out=tmp,
                            in0=src[:, jj * D : (jj + 1) * D],
                            scalar=0.0,
                            in1=kb,
                            op0=OP.bypass,
                            op1=OP.mult,
                            accum_out=eraw[:, j : j + 1],
                        )

                c0 = g * GRP
                m0 = b * NG * GRP + c0
                w = GRP // n_chains
                for ci in range(n_chains):
                    lo = ci * w
                    eij = small.tile([P, w], f32, name="eij", tag="eij")
                    nc.vector.tensor_add(
                        eij, eraw[:, lo : lo + w], bias_t[:, c0 + lo : c0 + lo + w]
                    )
                    th = small.tile([P, w], f32, name="th", tag="th")
                    nc.scalar.activation(th, eij, FT.Tanh, bias=zero_b)
                    ex = small.tile([P, w], f32, name="ex", tag="ex")
                    nc.scalar.activation(ex, th, FT.Exp, bias=zero_b)
                    a_m = small.tile([P, w], xdt, name="a_m", tag="a_m")
                    nc.gpsimd.tensor_mul(
                        a_m, ex, mask_all[:, m0 + lo : m0 + lo + w]
                    )

                    def den_mm():
                        nc.tensor.matmul(
                            den_ps[:, m0 + lo : m0 + lo + w],
                            lhsT=ones,
                            rhs=a_m,
                            start=True,
                            stop=True,
                        )

                    for jj2 in range(w):
                        j = lo + jj2
                        h, jj = divmod(j, cpt)
                        nc.tensor.matmul(
                            u_ps[b][:, :],
                            lhsT=a_m[:, jj2 : jj2 + 1],
                            rhs=xh[h][:, jj * D : (jj + 1) * D],
                            start=(g == 0 and j == 0),
                            stop=(g == NG - 1 and j == GRP - 1),
                        )
                    den_mm()

            pending_fin = None
            for b in range(BC):
                for g in range(NG):
                    if pending_fin is not None and g == 2:
                        _finalize(pending_fin)
                        pending_fin = None
                    last = b == BC - 1 and g == NG - 1
                    first = b == 0 and g == 0
                    # Tail drain: once no further x DMAs can be stalled,
                    # shift an extra column onto GpSimd+ACT so the DVE
                    # backlog clears with the last DMA.
                    tail = b == BC - 1 and g >= 2
                    emit_group(
                        b,
                        g,
                        (3, 6, 7) if tail else GPS_J_EVEN,
                        4 if last else 1,
                    )
                pending_fin = b
            _finalize(BC - 1)

    nc.compile()
    return nc


def _get_program(fp32r: bool):
    if fp32r not in _PROGRAM_CACHE:
        _PROGRAM_CACHE[fp32r] = _build_program(fp32r)
    return _PROGRAM_CACHE[fp32r]


def _prep_inputs(x, kern, bias, mask):
    """Host-side sharding/layout marshaling (views + tiny transposes only)."""
    x = np.ascontiguousarray(x, dtype=np.float32)
    kern = np.asarray(kern, dtype=np.float32)
    bias = np.asarray(bias, dtype=np.float32)
    kb = np.ascontiguousarray(kern[None, :])
    NM = NG // MEGA
    bias_t = np.ascontiguousarray(
        bias.reshape(NM, P, MEGA * GRP).transpose(1, 0, 2).reshape(P, NG * GRP)
    )
    mask_f = np.asarray(mask).astype(np.float32)
    in_maps = []
    for i in range(N_CORES):
        xs = x[i * BC : (i + 1) * BC].reshape(BC, NM, P, MEGA * GRP * D)
        ms = (
            mask_f[i * BC : (i + 1) * BC]
            .reshape(BC, NM, P, MEGA * GRP)
            .transpose(0, 2, 1, 3)
            .reshape(BC, P, NG * GRP)
        )
        in_maps.append(
            {
                "x": xs,
                "kb": kb,
                "bias_t": bias_t,
                "mask_t": np.ascontiguousarray(ms),
                "ones": np.ones((P, 1), dtype=np.float32),
            }
        )
    return in_maps


def kernel(x, kernel, bias, mask):
    global LAST_RESULTS
    nc = _get_program(PASS_B_FP32R)
    in_maps = _prep_inputs(x, kernel, bias, mask)
    res = run_bass_kernel_spmd(nc, in_maps, list(range(N_CORES)), trace=TRACE)
    LAST_RESULTS = res
    out = np.concatenate(
        [res.results[i]["out"].reshape(BC, D) for i in range(N_CORES)], axis=0
    )
    return out.astype(np.float32, copy=False)

